# revision 1
# baseline (speedup 1.0000x reference)
"""LocalizeAttention3D (3x3x3 neighborhood gather / im2col) Trainium2 kernel.

Reference op: x [b=2, h=8, n=13824, d=16] f32, n = 24*24*24 voxels (i,j,k)
-> out [b, h, n, 27, d] where out[., n=(i,j,k), f=(oi,oj,ok), :] =
   x[., (i+oi-1, j+oj-1, k+ok-1), :]  (zero outside the volume; filter index
   f = oi*9 + oj*3 + ok with oi,oj,ok in {0,1,2}).

Sharding: data-parallel over the 16 (b,h) pairs -> 2 per NeuronCore.

Per-core kernel (pure DMA expansion, memory-bound):
  * SBUF tile per (bh, group): partition p = flattened voxel-row r=(i*24+j)
    (72 valid rows per group + 25-row halo each side = 122 partitions),
    free dim = k-padded row [kpad=26, d=16] f32 (zeros at kpad 0 and 25).
  * For each (oi, oj) of the 9 in-plane shifts, a single 3-dim-AP DMA
    SBUF->HBM writes, for every (row, k), the 192 B output chunk holding the
    3 filters (oi, oj, ok=-1..1): source elem = 48 floats at kpad offset
    k*16 (overlapping window reads), dest elem = 48 floats at filter offset
    f0 = ((oi+1)*9 + (oj+1)*3)*16 of voxel n = r*24+k.
  * k-boundary zeros come from the SBUF kpad columns; i-boundary zeros from
    zeroed halo rows; j-boundary zeros are written by 12 small zero-fill
    DMAs into regions no shift-DMA touches (all output writes are disjoint,
    so no DRAM write-ordering is relied upon).
"""

import numpy as np

B, H_HEADS = 2, 8
HWD = 24  # height = width = depth
NVOX = HWD * HWD * HWD  # 13824
D = 16
NF = 27
NCORES = 8
BH_PER_CORE = (B * H_HEADS) // NCORES  # 2

ROWS = HWD * HWD  # 576 voxel-rows (i,j) per volume
K = HWD  # 24
KP = K + 2  # k-padded row length
ROWF = KP * D  # 416 floats per partition-row
GROUPS = 8
RV = ROWS // GROUPS  # 72 valid rows per group
HALO = HWD + 1  # 25: max |24*oi + oj| shift

_CACHE = {}


def _build_nc():
    from concourse import bacc, mybir
    import concourse.bass as bass
    import concourse.tile as tile

    nc = bacc.Bacc("TRN2", target_bir_lowering=False, debug=False)

    x = nc.dram_tensor("x", [BH_PER_CORE, NVOX, D], mybir.dt.float32,
                       kind="ExternalInput")
    out = nc.dram_tensor("out", [BH_PER_CORE, NVOX, NF, D], mybir.dt.float32,
                         kind="ExternalOutput")

    XS = NVOX * D          # x floats per bh
    OS = NVOX * NF * D     # out floats per bh
    VOXF = NF * D          # 432 floats per output voxel
    ROWOF = K * VOXF       # 10368 floats per out voxel-row
    XROWF = K * D          # 384 floats per input voxel-row

    f32 = mybir.dt.float32
    engines = [nc.sync, nc.scalar]
    _eng_i = [0]

    def eng():
        e = engines[_eng_i[0] % len(engines)]
        _eng_i[0] += 1
        return e

    with tile.TileContext(nc) as tc:
        with tc.tile_pool(name="zeros", bufs=1) as zpool, \
             tc.tile_pool(name="vol", bufs=4) as vpool:
            ztile = zpool.tile([24, 24 * 48], f32)
            nc.vector.memset(ztile[:, :], 0.0)
            zt = ztile.tensor

            for bh in range(BH_PER_CORE):
                for g in range(GROUPS):
                    r0 = g * RV  # first valid row of this group
                    vt = vpool.tile([128, ROWF], f32, name=f"vt_{bh}_{g}")
                    t = vt.tensor
                    nc.vector.memset(vt[:, :], 0.0)

                    # load rows [rlo, rhi) into partitions starting p_lo
                    rlo = max(0, r0 - HALO)
                    rhi = min(ROWS, r0 + RV + HALO)
                    p_lo = rlo - (r0 - HALO)
                    nrows = rhi - rlo
                    eng().dma_start(
                        out=bass.AP(t, p_lo * ROWF + D,
                                    [[ROWF, nrows], [1, XROWF]]),
                        in_=bass.AP(x, bh * XS + rlo * XROWF,
                                    [[XROWF, nrows], [1, XROWF]]),
                    )

                    for oi in (-1, 0, 1):
                        for oj in (-1, 0, 1):
                            dlt = 24 * oi + oj
                            f0 = (oi + 1) * 9 + (oj + 1) * 3
                            if oj == 0:
                                # all 72 rows in one DMA
                                eng().dma_start(
                                    out=bass.AP(
                                        out,
                                        bh * OS + r0 * ROWOF + f0 * D,
                                        [[ROWOF, RV], [VOXF, K], [1, 48]]),
                                    in_=bass.AP(
                                        t, (HALO + dlt) * ROWF,
                                        [[ROWF, RV], [D, K], [1, 48]]),
                                )
                            else:
                                # skip rows whose j+oj is out of bounds:
                                # per i-block (24 rows), 23 valid rows
                                jlo = 1 if oj < 0 else 0
                                for blk in range(RV // 24):
                                    rb = r0 + blk * 24 + jlo
                                    p0 = HALO + (rb - r0) + dlt
                                    eng().dma_start(
                                        out=bass.AP(
                                            out,
                                            bh * OS + rb * ROWOF + f0 * D,
                                            [[ROWOF, 23], [VOXF, K], [1, 48]]),
                                        in_=bass.AP(
                                            t, p0 * ROWF,
                                            [[ROWF, 23], [D, K], [1, 48]]),
                                    )

            # j-boundary zeros: rows j=0 (for oj=-1) and j=23 (for oj=+1)
            for bh in range(BH_PER_CORE):
                for oi in (-1, 0, 1):
                    for oj in (-1, 1):
                        f0 = (oi + 1) * 9 + (oj + 1) * 3
                        jz = 0 if oj < 0 else HWD - 1
                        eng().dma_start(
                            out=bass.AP(
                                out,
                                bh * OS + jz * ROWOF + f0 * D,
                                [[24 * ROWOF, 24], [VOXF, K], [1, 48]]),
                            in_=bass.AP(
                                zt, 0,
                                [[24 * 48, 24], [48, K], [1, 48]]),
                        )

    nc.compile()
    return nc


def _get_nc():
    if "nc" not in _CACHE:
        _CACHE["nc"] = _build_nc()
    return _CACHE["nc"]


def kernel(x, height=None, width=None, depth=None, **_kw):
    from concourse.bass_utils import run_bass_kernel_spmd

    x = np.ascontiguousarray(np.asarray(x), dtype=np.float32)
    b, h, n, d = x.shape
    assert (b, h, n, d) == (B, H_HEADS, NVOX, D), x.shape

    xs = x.reshape(b * h, n, d)
    in_maps = [
        {"x": np.ascontiguousarray(xs[c * BH_PER_CORE:(c + 1) * BH_PER_CORE])}
        for c in range(NCORES)
    ]
    res = run_bass_kernel_spmd(_get_nc(), in_maps, list(range(NCORES)))
    full = np.concatenate([res.results[c]["out"] for c in range(NCORES)], axis=0)
    return np.ascontiguousarray(full.reshape(b, h, n, NF, d).astype(np.float32))
